# revision 57
# baseline (speedup 1.0000x reference)
"""Trainium2 Bass kernel for ConvTemporalGraphical-style gated graph conv.

Computation (see reference):
    g   = x.reshape(N, F)                       # F = C*T*V = 204800
    h0  = elu(g @ W0 + b0)                      # [N, 256]   <-- dominant cost
    h1  = elu(h0 @ W1 + b1)                     # [N, 256]
    w   = softmax(h1 @ W2 + b2)                 # [N, 4]
    AS  = einsum('ne,etvw->ntvw', w, A)         # [N, T, V, V]
    out = einsum('nctv,ntvw->nctw', x, AS)

Sharding across 8 NeuronCores (one chip):
  * The F (contraction) dim of the big gating matmul is split 8 ways: core c
    holds W0 rows [c*25600, (c+1)*25600) and the matching slice of x,
    producing a partial h0 [32, 256] (fp8 DoubleRow matmuls, fp32 PSUM).
  * A ReduceScatter (32KB in, 4KB out) both combines the partials and hands
    each core exactly its own 4 samples' h0 rows — each core only needs the
    gate weights w[n] for the samples whose graph conv it owns, so the small
    MLP + softmax then runs on [4, 256] locally.  (ReduceScatter is ~2x
    cheaper than AllReduce end-to-end: no second distribution pass.)
  * The mixture + graph conv is data-parallel: core c owns samples [4c, 4c+4).

Device-friendly input layouts are produced on the host while sharding:
  * xgT: the gating x slice pre-transposed to [128, 200, 32] fp8 k-chunks
    (contraction dim on partitions), so no on-device transposes are needed.
  * W0s: fp8 [128, 200, 256] partition-major k-chunks, DMA'd as contiguous
    5120B lines and fed straight into DoubleRow (k-pair) matmuls.
  * xcT / A4p: conv-side tensors pre-arranged into a v-padded layout
    (partition = 32*b + v with t = 32*b + g) so the 25x25(x64) graph-conv
    matmuls can be packed 4-at-a-time into the PE array via `tile_position`
    row groups, with samples paired on PSUM partition halves (col groups)
    so output DMAs use all 128 partitions.
  * The conv-side loads (xcT, A4p) are gated behind the end of the W0 DMA
    stream (tiny guard copy) so they never steal HBM/DMA slots from the
    critical W0 stream; they land in the ReduceScatter's idle window.
  * Output is written bf16 (host upcasts to fp32): halves the final DMA and
    the PSUM->SBUF copy cost; adds <=2^-9 relative rounding on the output.
"""

import sys

if "/opt/trn_rl_repo" not in sys.path:
    sys.path.insert(0, "/opt/trn_rl_repo")

import numpy as np

import concourse.bass as bass
import concourse.mybir as mybir
import concourse.tile as tile
from concourse import bacc
from concourse import bass_utils
from concourse.masks import make_identity

# Problem dims (hardcoded per contract).
N, C, T, V = 32, 64, 128, 25
F = C * T * V            # 204800
H = 256
E = 4
NCORES = 8
KS = F // NCORES         # 25600 rows of W0 per core
NLOC = N // NCORES       # 4 samples per core (conv slice)
KCH = KS // 128          # 200 k-chunks of 128 per core
TG = T // 4              # 32 t-groups; t = 32*b + g (b = row block, g = group)
W0GRP = 10               # k-chunks per W0 load

FP32 = mybir.dt.float32
BF16 = mybir.dt.bfloat16
AX = mybir.AxisListType
ALU = mybir.AluOpType
ACTF = mybir.ActivationFunctionType

CFG = {
    "gating_dtype": "fp8",    # fp8 (e4m3, W0 scaled by 256 on host) halves
                              # the dominant W0 HBM stream vs bf16 and
                              # enables DoubleRow (2 k-chunks/instruction).
    "conv_dtype": "bf16",     # conv feeds the output; bf16 rel err ~4e-3.
    "double_row": True,       # fp8 DoubleRow gating matmuls (0.5 cyc/row)
    "collective": "rs",       # "rs" | "ar": ReduceScatter is ~15us in the
                              # cost model vs ~30us for AllReduce (1.875x).
    "out_bf16": True,         # bf16 output + host upcast: halves out DMA.
    "delay_conv_loads": True, # gate xcT/A4p DMA behind the W0 stream end.
}

W0_FP8_SCALE = 256.0          # W0 ~ U(+-1/452); scale into e4m3 normal range
FP8 = mybir.dt.float8e4


def _gdt():
    d = CFG["gating_dtype"]
    return FP8 if d == "fp8" else (BF16 if d == "bf16" else FP32)


def _cdt():
    return BF16 if CFG["conv_dtype"] == "bf16" else FP32


def _odt():
    return BF16 if CFG["out_bf16"] else FP32


def build():
    nc = bacc.Bacc("TRN2", target_bir_lowering=False, debug=False, num_devices=NCORES)

    gdt = _gdt()
    xgT = nc.dram_tensor("xgT", [128, KCH, N], gdt, kind="ExternalInput")
    xcT = nc.dram_tensor("xcT", [128, NLOC // 2, TG, 2 * C], _cdt(),
                         kind="ExternalInput")
    # W0 pre-transposed on host to partition-major [128, KCH, H] so every
    # group DMA moves 128 x (W0GRP*H) fully contiguous lines.
    W0s = nc.dram_tensor("W0s", [128, KCH, H], gdt, kind="ExternalInput")
    # b0 pre-scaled on host (x W0_FP8_SCALE / NCORES) and cast to the gating
    # dtype; folded into the gating matmul as a K=1 ones x b0 accumulation.
    b0 = nc.dram_tensor("b0g", [1, H], gdt, kind="ExternalInput")
    W1 = nc.dram_tensor("W1", [H, H], FP32, kind="ExternalInput")
    b1 = nc.dram_tensor("b1", [H], FP32, kind="ExternalInput")
    W2 = nc.dram_tensor("W2", [H, E], FP32, kind="ExternalInput")
    b2 = nc.dram_tensor("b2", [E], FP32, kind="ExternalInput")
    A4p = nc.dram_tensor("A4p", [128, E, TG * V], _cdt(), kind="ExternalInput")
    out = nc.dram_tensor("out", [NLOC, C, T * V], _odt(), kind="ExternalOutput")

    with tile.TileContext(nc) as tc:
        _build_body(nc, tc, xgT, xcT, W0s, b0, W1, b1, W2, b2, A4p, out)
    nc.compile()
    return nc


def _build_body(nc, tc, xgT, xcT, W0s, b0, W1, b1, W2, b2, A4p, out):
    from contextlib import ExitStack

    def _as_ap(t):
        return t if isinstance(t, bass.AP) else t.ap()

    xgT, xcT, W0s, b0, W1, b1, W2, b2, A4p, out = map(
        _as_ap, (xgT, xcT, W0s, b0, W1, b1, W2, b2, A4p, out)
    )
    gdt = _gdt()
    cdt = _cdt()
    odt = _odt()
    use_rs = CFG["collective"] == "rs"
    NW = NLOC if use_rs else N      # samples the post-collective MLP sees

    ctx = ExitStack()
    with ctx:
        const = ctx.enter_context(tc.tile_pool(name="const", bufs=1))
        w0_pool = ctx.enter_context(tc.tile_pool(name="w0_pool", bufs=10))
        mix_pool = ctx.enter_context(tc.tile_pool(name="mix_pool", bufs=2))
        out_pool = ctx.enter_context(tc.tile_pool(name="out_pool", bufs=2))
        dram = ctx.enter_context(tc.tile_pool(name="dram", bufs=1, space="DRAM"))
        # PSUM bank budget (8): pg 1 + ph 1 + pc 6 (po0/po1 double-buffered)
        pg = ctx.enter_context(tc.tile_pool(name="pg", bufs=1, space="PSUM"))
        ph = ctx.enter_context(tc.tile_pool(name="ph", bufs=1, space="PSUM"))
        pc = ctx.enter_context(tc.tile_pool(name="pc", bufs=1, space="PSUM"))

        # ---- persistent big SBUF tensors ----
        xT_all = const.tile([128, KCH, N], gdt)           # gating x^T chunks
        xcT_all = const.tile([128, NLOC // 2, TG, 2 * C], cdt)
        A_sb = const.tile([128, E, TG * V], cdt)          # padded A
        AS_sb = const.tile([128, NLOC, TG * V], cdt)      # mixture output

        # ---- bulk input loads (pre-transposed / pre-padded on host) ----
        # The W0 stream is striped over both HWDGE queues (SP/Act) plus the
        # Pool SWDGE queue (slower per descriptor, so it gets a smaller
        # share) — the queues transfer concurrently, so the 6.5MB fp8
        # stream drains in well under half the single-queue time.  xgT
        # quarters interleave (quarter q covers k-chunks [50q, 50q+50)).
        Q = KCH // 4
        # 20 W0 groups: SP x8, Act x6, Pool x6 (Act starts late behind the
        # 1.3us activation-table load; Pool desc-gen is ~5% slower per group)
        w0_qs = [
            nc.sync, nc.scalar, nc.gpsimd, nc.sync, nc.scalar,
            nc.gpsimd, nc.sync, nc.scalar, nc.gpsimd, nc.sync,
            nc.scalar, nc.gpsimd, nc.sync, nc.scalar, nc.gpsimd,
            nc.sync, nc.scalar, nc.gpsimd, nc.sync, nc.sync,
        ]

        def load_xq(q, eng):
            eng.dma_start(
                xT_all[:, q * Q:(q + 1) * Q, :], xgT[:, q * Q:(q + 1) * Q, :]
            )

        load_xq(0, nc.sync)
        load_xq(1, nc.scalar)
        load_xq(2, nc.gpsimd)
        b0g_sb = const.tile([1, H], gdt)
        nc.gpsimd.dma_start(b0g_sb[:], b0[:])
        ones_col = const.tile([1, N], gdt)
        nc.gpsimd.memset(ones_col[:], 1.0)

        # =========================================================
        # Gating matmul: fp32-accumulated fp8 DoubleRow matmuls (2 k-chunks
        # per instruction) over the 200-chunk local k-range.
        # =========================================================
        h0_ps = pg.tile([N, H], FP32, tag="h0ps", name="h0ps")
        use_dr = CFG["double_row"] and CFG["gating_dtype"] == "fp8"
        kstep = 2 if use_dr else 1
        pm = mybir.MatmulPerfMode.DoubleRow if use_dr else None

        w0_last = None
        NG = KCH // W0GRP
        for g in range(NG):
            w0_t = w0_pool.tile([128, W0GRP, H], gdt, tag="w0_t")
            w0_src = W0s[:, g * W0GRP:(g + 1) * W0GRP, :]
            if g == NG - 1:
                # split the final group 6+4 so the last-arriving chunk is
                # small and the matmul drain after the stream is short
                w0_qs[g].dma_start(w0_t[:, :6], w0_src[:, :6])
                w0_qs[g].dma_start(w0_t[:, 6:], w0_src[:, 6:])
            else:
                w0_qs[g].dma_start(w0_t[:], w0_src)
            w0_last = w0_t
            if g == 5:
                load_xq(3, nc.gpsimd)
            for j in range(0, W0GRP, kstep):
                k = g * W0GRP + j
                if use_dr:
                    nc.tensor.matmul(
                        h0_ps[:],
                        xT_all[:, k:k + 2, :],
                        w0_t[:, j:j + 2, :],
                        start=(k == 0),
                        stop=False,
                        perf_mode=pm,
                    )
                else:
                    nc.tensor.matmul(
                        h0_ps[:],
                        xT_all[:, k, :],
                        w0_t[:, j, :],
                        start=(k == 0),
                        stop=False,
                    )
        # fold b0 into the accumulation (K=1 ones x b0g row)
        nc.tensor.matmul(
            h0_ps[:], ones_col[:], b0g_sb[:], start=False, stop=True,
        )

        # constants + small-weight loads, emitted late so they fill DMA gaps
        # rather than delaying the W0 stream.
        identity = const.tile([128, 128], FP32)
        make_identity(nc, identity)

        ones_row = const.tile([1, NW], FP32)
        nc.gpsimd.memset(ones_row[:], 1.0)
        ones_np = const.tile([NW, 128], FP32)
        nc.gpsimd.memset(ones_np[:], 1.0)
        b1_row = const.tile([1, H], FP32)
        b2_row = const.tile([1, E], FP32)
        W1_sb = const.tile([128, 2, H], FP32)
        W2_sb = const.tile([128, 2, E], FP32)

        # selector constant for the wbd block-diagonal product, built once
        # off the critical path: Isel[k, nl, e] = identity[k, nl]
        ones4e = const.tile([NW, E], FP32)
        nc.gpsimd.memset(ones4e[:], 1.0)
        Isel = const.tile([NW, NLOC, E], FP32)
        for nl in range(NLOC):
            nc.vector.tensor_scalar(
                Isel[:, nl, :], ones4e[:], identity[:NW, nl:nl + 1], None,
                ALU.mult,
            )

        # =========================================================
        # Cross-core reduction.  ReduceScatter leaves core c holding the
        # summed h0 rows for exactly its 4 samples [4c, 4c+4).
        # cc_in goes out on the DVE queue so it is not stuck behind the
        # conv-side loads on SP/Act.
        # =========================================================
        hp = const.tile([N, H], FP32, name="h0p")
        nc.vector.tensor_copy(hp[:], h0_ps[:])

        # All remaining input loads are gated behind the end of the gating
        # matmul via 1-element guard copies from hp (WAR dep on each DMA
        # dst).  The DMA queues schedule by readiness, so without the guards
        # these transfers get hoisted into (and lengthen) the W0 stream;
        # with them they run in the collective's idle window.  cc_in's DMA
        # becomes ready a hair earlier (it waits on hp directly), so it
        # drains first.
        if CFG["delay_conv_loads"]:
            for gt in (A_sb[:1, 0, :1], xcT_all[:1, 0, 0, :1],
                       xcT_all[:1, 1, 0, :1], b1_row[:1, :1], b2_row[:1, :1],
                       W1_sb[:1, 0, :1], W2_sb[:1, 0, :1]):
                nc.gpsimd.tensor_copy(gt, hp[:1, :1])

        cc_in = dram.tile([N, H], FP32, name="cc_in")
        cc_out = dram.tile(
            [NW, H], FP32, name="cc_out",
            **({} if use_rs else {"addr_space": "Shared"}),
        )
        nc.sync.dma_start(cc_in[:], hp[:])
        nc.gpsimd.collective_compute(
            "ReduceScatter" if use_rs else "AllReduce",
            ALU.add,
            replica_groups=[list(range(NCORES))],
            ins=[cc_in.opt()],
            outs=[cc_out.opt()],
        )

        nc.sync.dma_start(A_sb[:], A4p[:])
        nc.sync.dma_start(xcT_all[:, 0], xcT[:, 0])
        nc.scalar.dma_start(xcT_all[:, 1], xcT[:, 1])

        # softmax weights sum to 1, so AS[n] = A0 + sum_{e>=1} w[n,e]*D_e
        # with D_e = A_e - A0: a 3-scale mixture instead of 4.  The deltas
        # are computed in the collective's idle window (DVE/Pool are idle).
        D_sb = const.tile([128, 3, TG * V], cdt)
        for e in (1, 2, 3):
            eng = (nc.vector, nc.gpsimd, nc.vector)[e - 1]
            eng.tensor_tensor(
                D_sb[:, e - 1, :], A_sb[:, e, :], A_sb[:, 0, :], ALU.subtract
            )
        nc.gpsimd.dma_start(b1_row[:], b1.rearrange("(o h) -> o h", o=1))
        nc.sync.dma_start(b2_row[:], b2.rearrange("(o h) -> o h", o=1))
        nc.gpsimd.dma_start(W1_sb[:], W1.rearrange("(j p) h -> p j h", p=128))
        nc.scalar.dma_start(W2_sb[:], W2.rearrange("(j p) h -> p j h", p=128))

        # read the local h0 rows back TRANSPOSED ([h%128, h//128, n]) so the
        # h1 matmuls consume them directly — no PE transpose round-trip.
        # One strided DMA per h-half (the DMA AP balancer caps at 3 dims).
        h0T = const.tile([128, 2, NW], FP32)
        cc_outT = cc_out.rearrange("n h -> h n")
        nc.sync.dma_start(h0T[:, 0, :], cc_outT[0:128, :])
        nc.scalar.dma_start(h0T[:, 1, :], cc_outT[128:256, :])

        # =========================================================
        # Tiny MLP + softmax on the NW local samples
        # =========================================================
        # b0 was folded into the gating matmul (ones x b0g k-row).  The fp8
        # 1/256 rescale is fused into elu's min/max tensor_scalars, and the
        # elu "-1" is folded into b1 on the host (b1_eff = b1 - W1.sum(0)),
        # so the whole h0 epilogue is 4 ops:
        #   tmp = exp(min(s*h0, 0));  h0 = max(s*h0, 0);  h0 += tmp
        s_h0 = 1.0 / W0_FP8_SCALE if CFG["gating_dtype"] == "fp8" else 1.0
        h0Tf = h0T[:].rearrange("p j n -> p (j n)")
        elu0_tmp = const.tile([128, 2 * NW], FP32)
        nc.vector.tensor_scalar(elu0_tmp[:], h0Tf, s_h0, 0.0, ALU.mult, ALU.min)
        nc.scalar.activation(elu0_tmp[:], elu0_tmp[:], ACTF.Exp)
        nc.vector.tensor_scalar(h0Tf, h0Tf, s_h0, 0.0, ALU.mult, ALU.max)
        nc.vector.tensor_tensor(h0Tf, h0Tf, elu0_tmp[:], ALU.add)

        # PE warm-up: the PE p-state ramp resets when the engine idles (it
        # idles through the whole collective), so the conv would run at the
        # mid p-state.  Garbage matmuls (into the dead gating PSUM bank)
        # interleaved with the MLP keep the PE continuously busy from the
        # collective's end until the conv starts, so the conv runs at full
        # speed.  They read h0T so they cannot start before the collective.
        warm_ps = pg.tile([N, H], FP32, tag="h0ps", name="warm_ps")

        def warm(k, width=128):
            for _ in range(k):
                nc.tensor.matmul(
                    warm_ps[:2 * NW, :width],
                    h0T[:].rearrange("p j n -> p (j n)"),
                    identity[:, :width],
                    start=True, stop=True, skip_group_check=True,
                )

        # h1 computed TRANSPOSED (h1T[h', n]) so it feeds the logits matmul
        # as lhsT with no second transpose pair.
        h1T_ps = ph.tile([128, 2 * NW], FP32, tag="mlp_ps")
        for mj in range(2):
            for kj in range(2):
                nc.tensor.matmul(
                    h1T_ps[:, mj * NW:(mj + 1) * NW],
                    W1_sb[:, kj, mj * 128:(mj + 1) * 128],
                    h0T[:, kj, :],
                    start=(kj == 0),
                    stop=False,
                )
            nc.tensor.matmul(
                h1T_ps[:, mj * NW:(mj + 1) * NW],
                b1_row[:, mj * 128:(mj + 1) * 128],
                ones_row[:],
                start=False,
                stop=True,
            )
        warm(2)
        # elu on h1T with the PSUM->SBUF copy folded into the min/max
        # tensor_scalars (both read h1T_ps directly).
        h1T = const.tile([128, 2, NW], FP32)
        h1Tf = h1T[:].rearrange("p j n -> p (j n)")
        elu1_tmp = const.tile([128, 2 * NW], FP32)
        nc.vector.tensor_scalar(elu1_tmp[:], h1T_ps[:], 0.0, None, ALU.min)
        nc.scalar.activation(elu1_tmp[:], elu1_tmp[:], ACTF.Exp)
        nc.vector.tensor_scalar(h1Tf, h1T_ps[:], 0.0, -1.0, ALU.max, ALU.add)
        nc.vector.tensor_tensor(h1Tf, h1Tf, elu1_tmp[:], ALU.add)

        lg_ps = ph.tile([NW, E], FP32, tag="mlp_ps")
        for j in range(2):
            nc.tensor.matmul(
                lg_ps[:], h1T[:, j, :], W2_sb[:, j, :],
                start=(j == 0), stop=False,
            )
        nc.tensor.matmul(
            lg_ps[:], ones_row[:], b2_row[:], start=False, stop=True,
        )
        warm(2)

        # softmax over E (free dim); logits are bounded (|x| < ~2: elu-bounded
        # h1 times U(+-1/16) weights over K=256), so skip the max-subtraction.
        ex = const.tile([NW, E], FP32)
        sm = const.tile([NW, 1], FP32)
        nc.scalar.activation(ex[:], lg_ps[:], ACTF.Exp, accum_out=sm[:])
        rec = const.tile([NW, 1], FP32)
        nc.vector.reciprocal(rec[:], sm[:])

        # Local-w 128-partition broadcast in one matmul: wbd[k, nl*E+e] =
        # ex[k, e] * I[k, nl] (block-diagonal product), then recb^T @ wbd
        # sums over k with recb[k, p] = 1/sum[k] — the softmax normalization
        # rides the broadcast matmul's stationary operand, so the wbd
        # product runs in PARALLEL with the reciprocal instead of after it.
        # With ReduceScatter the local rows ARE the owned samples, so the
        # selector is just the identity.
        assert use_rs, "AR fallback needs a per-core sample selector (removed)"
        wbd = const.tile([NW, NLOC, E], FP32)
        nc.vector.tensor_tensor(
            wbd[:],
            ex[:].unsqueeze(1).broadcast_to([NW, NLOC, E]),
            Isel[:],
            ALU.mult,
        )
        recb = const.tile([NW, 128], FP32)
        nc.vector.tensor_scalar(recb[:], ones_np[:], rec[:], None, ALU.mult)
        wb_ps = ph.tile([128, NLOC * E], FP32, tag="mlp_ps")
        nc.tensor.matmul(
            wb_ps[:], recb[:],
            wbd[:].rearrange("k nl e -> k (nl e)"),
            start=True, stop=True,
        )
        warm(5)
        w_bcast = const.tile([128, NLOC * E], FP32)
        nc.vector.tensor_copy(w_bcast[:], wb_ps[:])

        # =========================================================
        # Mixture AS[n] = sum_e ex[n,e] * A[e] interleaved with the graph
        # conv per sample-pair, so conv pair 0 starts as soon as AS[0..1]
        # are ready.  tensor_scalar (4x on DVE for packed bf16) + plain
        # tensor_tensor joins; scalar_tensor_tensor is NOT used (no DVE
        # perf modes).  Work is spread over DVE/Act/Pool.
        # =========================================================
        def emit_mixture(n):
            wp = lambda e: wb_ps[:, n * E + e:n * E + e + 1]    # PSUM direct
            ws = lambda e: w_bcast[:, n * E + e:n * E + e + 1]  # SBUF (Act)
            t1 = mix_pool.tile([128, TG * V], cdt, tag="mix_t1", name="t1")
            t2 = mix_pool.tile([128, TG * V], cdt, tag="mix_t2", name="t2")
            t3 = mix_pool.tile([128, TG * V], cdt, tag="mix_t3", name="t3")
            # DVE may read the PSUM scalars directly; GPSIMD/Act must not
            # touch PSUM on real HW, so they read the SBUF copy.
            nc.vector.tensor_scalar(t1[:], D_sb[:, 0, :], wp(1), None, ALU.mult)
            nc.scalar.activation(t2[:], D_sb[:, 1, :], ACTF.Copy, scale=ws(2))
            nc.gpsimd.tensor_scalar(t3[:], D_sb[:, 2, :], ws(3), None, ALU.mult)
            nc.vector.tensor_tensor(t1[:], t1[:], t2[:], ALU.add)
            nc.gpsimd.tensor_tensor(t3[:], t3[:], A_sb[:, 0, :], ALU.add)
            nc.vector.tensor_tensor(AS_sb[:, n, :], t1[:], t3[:], ALU.add)

        def emit_conv_pair(pr):
            # PSUM -> SBUF copies (DMA cannot read PSUM) spread over
            # DVE/Act/Pool so they do not fight the mixture for one engine;
            # each (t-window, b) block's output DMA fires right after its
            # copy instead of waiting for the whole pair.
            ot = out_pool.tile([128, T * V], odt, tag="ot", name="ot")
            od = out[2 * pr:2 * pr + 2].rearrange("n c f -> (n c) f")
            for g0, glen in ((0, 20), (20, 12)):
                # width padded to 512 so the row stride is bank-aligned
                pob = [
                    pc.tile([128, 512], FP32, tag=f"po{b}", name=f"po{b}",
                            bufs=2 if b < 2 else 1)
                    for b in range(4)
                ]
                # j-major: all of sample 2pr's matmuls are emitted before
                # sample 2pr+1's, so the conv starts as soon as AS[2pr] is
                # mixed instead of waiting for both samples of the pair.
                # Within each j, the single-buffered b2/b3 banks are emitted
                # last so pair 1's b0/b1 matmuls are not head-of-line
                # blocked waiting for pair 0's b2/b3 drain.
                for j in range(2):
                    n = 2 * pr + j
                    for bpass in ((0, 1), (2, 3)):
                        for gi in range(glen):
                            g = g0 + gi
                            for b in bpass:
                                nc.tensor.matmul(
                                    pob[b][64 * j:64 * (j + 1),
                                           gi * V:(gi + 1) * V],
                                    xcT_all[32 * b:32 * b + V, pr, g,
                                            64 * j:64 * (j + 1)],
                                    AS_sb[32 * b:32 * b + V, n,
                                          g * V:(g + 1) * V],
                                    start=True,
                                    stop=True,
                                    tile_position=(32 * b, 64 * j),
                                )
                # PSUM->SBUF copies only on DVE/Act (GPSIMD cannot touch
                # PSUM on real hardware); each block's output DMA fires
                # right after its copy.  The very last window's odd blocks
                # (the kernel's final dependency) are copied in half-width
                # pieces on BOTH engines so their DMAs fire sooner.
                width = glen * V
                last = pr == 1 and g0 == 20
                for b in range(4):
                    lo = (32 * b + g0) * V
                    dst = ot[:, lo:lo + width]
                    if last and b % 2 == 1:
                        hw_ = width // 2
                        nc.vector.tensor_copy(
                            ot[:, lo:lo + hw_], pob[b][:, :hw_]
                        )
                        nc.scalar.activation(
                            ot[:, lo + hw_:lo + width], pob[b][:, hw_:width],
                            ACTF.Copy,
                        )
                    elif b % 2 == 0:
                        nc.vector.tensor_copy(dst, pob[b][:, :width])
                    else:
                        nc.scalar.activation(dst, pob[b][:, :width], ACTF.Copy)
                    eng = nc.sync if b % 2 == 0 else nc.scalar
                    eng.dma_start(od[:, lo:lo + width], ot[:, lo:lo + width])

        # All four mixtures are emitted before the conv pairs: the engine
        # queues are in-order, so putting pair 0's PSUM->SBUF copies ahead
        # of mixtures 2/3 would stall those mixtures (and thus pair 1's
        # matmuls) behind copies that wait on pair 0's matmuls.
        emit_mixture(0)
        emit_mixture(1)
        emit_mixture(2)
        emit_mixture(3)
        emit_conv_pair(0)
        emit_conv_pair(1)


_NC_CACHE = {}


def _get_nc():
    key = tuple(sorted(CFG.items()))
    if key not in _NC_CACHE:
        _NC_CACHE[key] = build()
    return _NC_CACHE[key]


def _to_bf16(a):
    """Round-to-nearest-even fp32 -> bf16, vectorized."""
    import ml_dtypes

    u = np.ascontiguousarray(a, dtype=np.float32).view(np.uint32)
    r = ((u + 0x7FFF + ((u >> 16) & 1)) >> 16).astype(np.uint16)
    return r.view(ml_dtypes.bfloat16)


def _shard_inputs(x, W0, b0, W1, b1, W2, b2, A):
    x = np.ascontiguousarray(np.asarray(x, dtype=np.float32))
    W0 = np.ascontiguousarray(np.asarray(W0, dtype=np.float32))
    A = np.ascontiguousarray(np.asarray(A, dtype=np.float32)).reshape(E, T, V, V)
    xf = x.reshape(N, F)
    gdtype = CFG["gating_dtype"]
    cbf16 = CFG["conv_dtype"] == "bf16"

    # A in padded layout: A4p[32b+v, e, g*V+w] = A[e, 32b+g, v, w]
    A4p = np.zeros((128, E, TG * V), dtype=np.float32)
    At = A.reshape(E, 4, TG, V, V)            # e b g v w
    for b in range(4):
        A4p[32 * b:32 * b + V, :, :] = (
            At[:, b].transpose(2, 0, 1, 3).reshape(V, E, TG * V)
        )

    A4p_cast = _to_bf16(A4p) if cbf16 else A4p
    in_maps = []
    for c in range(NCORES):
        # gating slice, pre-transposed to [128, KCH, N]
        xg = xf[:, c * KS:(c + 1) * KS]                   # [N, KS]
        xgT = np.ascontiguousarray(
            xg.reshape(N, KCH, 128).transpose(2, 1, 0)    # [128, KCH, N]
        )
        # W0 slice in partition-major layout [128, KCH, H]:
        # W0T[p, k, h] = W0[c*KS + k*128 + p, h]
        W0c = np.ascontiguousarray(
            W0[c * KS:(c + 1) * KS].reshape(KCH, 128, H).transpose(1, 0, 2)
        )
        if gdtype == "fp8":
            import ml_dtypes

            xgT = np.ascontiguousarray(xgT.astype(ml_dtypes.float8_e4m3))
            W0c = np.ascontiguousarray(
                (W0c * W0_FP8_SCALE).astype(ml_dtypes.float8_e4m3)
            )
        elif gdtype == "bf16":
            xgT = np.ascontiguousarray(_to_bf16(xgT))
            W0c = np.ascontiguousarray(_to_bf16(W0c))

        # conv slice, pre-transposed/padded:
        # xcT[32b+v, pr, g, 64j+cc] = x[4c + 2pr + j, cc, 32b+g, v]
        xl = x[c * NLOC:(c + 1) * NLOC]                   # [4, C, T, V]
        xcT = np.zeros((128, NLOC // 2, TG, 2 * C), dtype=np.float32)
        xr = xl.reshape(NLOC // 2, 2, C, 4, TG, V)        # pr j cc b g v
        for b in range(4):
            # [pr, j, cc, g, v] -> [v, pr, g, (j cc)]
            blk = xr[:, :, :, b]                          # pr j cc g v
            xcT[32 * b:32 * b + V] = (
                blk.transpose(4, 0, 3, 1, 2).reshape(V, NLOC // 2, TG, 2 * C)
            )

        b0v = np.asarray(b0, dtype=np.float32).reshape(1, H) / NCORES
        if gdtype == "fp8":
            import ml_dtypes

            b0g = np.ascontiguousarray(
                (b0v * W0_FP8_SCALE).astype(ml_dtypes.float8_e4m3)
            )
        elif gdtype == "bf16":
            b0g = np.ascontiguousarray(_to_bf16(b0v))
        else:
            b0g = b0v

        in_maps.append({
            "xgT": xgT,
            "xcT": _to_bf16(xcT) if cbf16 else xcT,
            "W0s": W0c,
            "b0g": b0g,
            "W1": np.asarray(W1, dtype=np.float32),
            # the device's h0-elu omits the "-1" (elu = max + exp - 1); the
            # constant -1 @ W1 is folded in here: b1_eff = b1 - W1.sum(0)
            "b1": np.asarray(b1, dtype=np.float32)
            - np.asarray(W1, dtype=np.float32).sum(axis=0),
            "W2": np.asarray(W2, dtype=np.float32),
            "b2": np.asarray(b2, dtype=np.float32),
            "A4p": A4p_cast,
        })
    return in_maps


def kernel(x, W0, b0, W1, b1, W2, b2, A):
    nc = _get_nc()
    in_maps = _shard_inputs(x, W0, b0, W1, b1, W2, b2, A)
    res = bass_utils.run_bass_kernel_spmd(nc, in_maps, core_ids=list(range(NCORES)))
    outs = [
        np.asarray(res.results[c]["out"], dtype=np.float32).reshape(NLOC, C, T, V)
        for c in range(NCORES)
    ]
    return np.concatenate(outs, axis=0)
